# revision 28
# baseline (speedup 1.0000x reference)
"""Causal attention (B=4, S=2048, D=1024, fp32) on 8 TRN2 NeuronCores.

Sharding: core c -> (batch b = c//2, key-parity h = c%2). Each core computes
q = x@Wq.T for all S queries of its batch, k/v only for key positions whose
128-block index has parity h (S/2 positions, causally load-balanced), then
scores^T = k q^T in [kpos, q] orientation (softmax denominator and A@V both
reduce over kpos = the PSUM contraction dim, so no on-chip transposes), and
returns the unnormalized partial output sum(exp(s)*v) plus the denominator
sum(exp(s)). Host adds the two partials per batch and divides. exp() is
computed without max-subtraction: scores*scale is ~N(0, 0.17) here, far from
fp32 overflow. All matmuls run as float32r (fp32 truncated inside the PE),
which streams at ~1 col/cycle warm for moving dims >= 256.

The parity-h key positions are extracted on-chip from the streamed x chunks
(no separate xk DMA). So the extraction slices are compile-time constants in
the shared SPMD NEFF, odd-parity cores receive x with adjacent 128-column
blocks swapped (query index XOR 128); their masks are built for the permuted
query order on the host and their pout/den rows are unpermuted on the host.
"""
import numpy as np

import concourse.bacc as bacc
import concourse.tile as tile
import concourse.mybir as mybir
from concourse import bass_utils
from concourse.tile import add_dep_helper
from contextlib import ExitStack

B, S, D = 4, 2048, 1024
QT = 256              # query tile
NT = S // QT          # 8 query tiles
SH = S // 2           # key positions per core
SCALE = 1.0 / 32.0    # 1/sqrt(D)
F32 = mybir.dt.float32
F32R = mybir.dt.float32r
BF16 = mybir.dt.bfloat16
EXP = mybir.ActivationFunctionType.Exp

_NC = None


def _dview(ap):
    """[D, C] dram tensor -> [128, 8, C] view (partition, d-block, col)."""
    return ap.rearrange("(d p) c -> p d c", p=128)


def _build():
    nc = bacc.Bacc()
    xT = nc.dram_tensor("xT", [D, S], F32, kind="ExternalInput").ap()
    wqT = nc.dram_tensor("wqT", [D, D], F32, kind="ExternalInput").ap()
    wkT = nc.dram_tensor("wkT", [D, D], F32, kind="ExternalInput").ap()
    wvT = nc.dram_tensor("wvT", [D, D], F32, kind="ExternalInput").ap()
    dmask = nc.dram_tensor("dmask", [2, 128, 512], F32, kind="ExternalInput").ap()
    pout = nc.dram_tensor("pout", [S, D], F32, kind="ExternalOutput").ap()
    den = nc.dram_tensor("den", [1, S], F32, kind="ExternalOutput").ap()

    def chain_to(inst, prev):
        add_dep_helper(inst.ins, prev.ins, sync=True, reason="input dma ordering")
        return inst

    with tile.TileContext(nc) as tc, ExitStack() as top:
        small = top.enter_context(tc.tile_pool(name="small", bufs=1))
        osb_pool = top.enter_context(tc.tile_pool(name="osb", bufs=2))
        qt_pool = top.enter_context(tc.tile_pool(name="qt", bufs=1))

        # q^T and k^T live as bf16: halves their SBUF footprint (so wk and
        # wv fit in fresh space with eager DMAs and no WAR stalls) and makes
        # the score matmuls 1-pass-LDWEIGHTS bf16 x bf16. The ~4e-3 relative
        # quantization on q/k perturbs the logits by ~1e-3 - far inside the
        # accuracy budget.
        qtt = qt_pool.tile([128, 8, S], BF16, name="qtt")
        qt = [qtt[:, e] for e in range(8)]
        ones_f = small.tile([128, 2], F32)
        ones = small.tile([128, 2], F32R)
        junk = small.tile([128, 512], F32R)
        den_sb = small.tile([2, 512], F32, name="den_sb")
        nc.vector.memset(ones_f, 1.0)
        nc.vector.tensor_copy(ones, ones_f)
        nc.vector.memset(junk.bitcast(F32), 0.0)
        nc.vector.tensor_copy(junk, junk)

        # xk ([d-part, d-block, parity-key-pos]) is filled from the streamed
        # x chunks during phase 1. wk and wv overlap the phase-1 pools'
        # lifetimes so the allocator gives them fresh space: their DMAs have
        # no WAR on the q-phase tiles and land during phase 1.
        attn = top.enter_context(ExitStack())
        xk_pool = attn.enter_context(tc.tile_pool(name="xk", bufs=1))
        xk = xk_pool.tile([128, 8, SH], F32R, name="xk")
        wk_pool = attn.enter_context(tc.tile_pool(name="wk", bufs=1))
        wk = wk_pool.tile([128, 8, D], F32R, name="wk")
        wv_pool = attn.enter_context(tc.tile_pool(name="wv", bufs=1))
        wv = wv_pool.tile([128, 8, D], F32R, name="wv")

        # ---- phase 1: q^T = Wq^T-contracted x^T, for all S queries ----
        # DMAs are emitted in consumption order on a single dependency
        # chain so HBM bandwidth follows the matmul stream: wq e0-slice +
        # x chunk 0 first (first PE group), then the wq tail paced against
        # the e-groups, then x chunks 1-3, then wk / wv / masks.
        with ExitStack() as ph:
            wq_pool = ph.enter_context(tc.tile_pool(name="wq", bufs=1))
            xs_pool = ph.enter_context(tc.tile_pool(name="xs", bufs=2))
            psB = ph.enter_context(tc.tile_pool(name="psB", bufs=6, space="PSUM"))
            warm_ps = ph.enter_context(tc.tile_pool(name="warm", bufs=1, space="PSUM"))
            wp = warm_ps.tile([128, 512], F32, name="wp")
            for _ in range(36):
                nc.tensor.matmul(wp[0:2, :], lhsT=junk[:, 0:2], rhs=junk,
                                 start=True, stop=True, skip_group_check=True)
            wq = wq_pool.tile([128, 8, D], F32R, name="wq")
            wqv = _dview(wqT.bitcast(F32R))
            xv = _dview(xT.bitcast(F32R))
            # strict-priority initial burst: only wq e0 + x chunk 0 (the
            # first PE group's inputs) race at t0; two chains hang off the
            # two x halves and deliver the wq tail, x chunks 1-3, and wk in
            # consumption order without stealing bandwidth from earlier
            # pieces.
            xs_tiles = [None] * (S // 512)
            xs_tiles[0] = xs_pool.tile([128, 8, 512], F32R, name="xs")
            # fine-grained initial burst: six 0.25 MiB transfers ramp six
            # queues in parallel so the first PE group's inputs (wq e0 +
            # x chunk 0) land as early as possible; the wq tail chains off
            # the x pieces and arrives well before the e4-e7 groups.
            nc.sync.dma_start(out=wq[:, :, 0:64], in_=wqv[:, :, 0:64])
            nc.sync.dma_start(out=wq[:, :, 64:128], in_=wqv[:, :, 64:128])
            x0 = [nc.sync.dma_start(out=xs_tiles[0][:, :, i * 128:(i + 1) * 128],
                                    in_=xv[:, :, i * 128:(i + 1) * 128])
                  for i in range(4)]
            x0a, x0b = x0[2], x0[3]
            wq1 = chain_to(nc.sync.dma_start(out=wq[:, :, 128:256], in_=wqv[:, :, 128:256]), x0[0])
            wq2 = chain_to(nc.sync.dma_start(out=wq[:, :, 256:512], in_=wqv[:, :, 256:512]), x0[1])
            wq3a = chain_to(nc.sync.dma_start(out=wq[:, :, 512:768], in_=wqv[:, :, 512:768]), x0[2])
            wq3b = chain_to(nc.sync.dma_start(out=wq[:, :, 768:D], in_=wqv[:, :, 768:D]), x0[3])
            xs_dma = [None] * (S // 512)
            pa, pb = wq3a, wq3b
            for c in range(1, S // 512):
                xs_tiles[c] = xs_pool.tile([128, 8, 512], F32R, name="xs")
                pa = chain_to(nc.sync.dma_start(
                    out=xs_tiles[c][:, :, 0:256],
                    in_=xv[:, :, c * 512:c * 512 + 256]), pa)
                pb = chain_to(nc.sync.dma_start(
                    out=xs_tiles[c][:, :, 256:512],
                    in_=xv[:, :, c * 512 + 256:(c + 1) * 512]), pb)
            iwk = chain_to(nc.sync.dma_start(
                out=wk, in_=_dview(wkT.bitcast(F32R))), pb)
            chain_to(nc.sync.dma_start(
                out=wv, in_=_dview(wvT.bitcast(F32R))), iwk)
            for c in range(S // 512):
                xs = xs_tiles[c]
                for e in range(8):
                    ps = psB.tile([128, 512], F32)
                    for d_ in range(8):
                        nc.tensor.matmul(ps, lhsT=wq[:, d_, e * 128:(e + 1) * 128],
                                         rhs=xs[:, d_, :], start=d_ == 0, stop=d_ == 7)
                    nc.vector.tensor_copy(qt[e][:, c * 512:(c + 1) * 512], ps)
                # extract this chunk's parity-h key positions (local column
                # blocks 0 and 2 after the host-side pair swap on h=1 cores)
                nc.scalar.copy(xk[:, :, c * 256:c * 256 + 128], xs[:, :, 0:128])
                nc.scalar.copy(xk[:, :, c * 256 + 128:c * 256 + 256], xs[:, :, 256:384])

        # ---- k projection; kt (bf16) goes into the freed wq/xs space ----
        kt_pool = attn.enter_context(tc.tile_pool(name="kt", bufs=1))
        ktt = kt_pool.tile([128, 8, SH], BF16, name="ktt")
        kt = [ktt[:, e] for e in range(8)]
        with ExitStack() as ph:
            psA = ph.enter_context(tc.tile_pool(name="psA", bufs=6, space="PSUM"))
            for sc in range(2):
                for e in range(8):
                    ps = psA.tile([128, 512], F32)
                    for d_ in range(8):
                        nc.tensor.matmul(
                            ps, lhsT=wk[:, d_, e * 128:(e + 1) * 128],
                            rhs=xk[:, d_, sc * 512:(sc + 1) * 512],
                            start=d_ == 0, stop=d_ == 7)
                    nc.vector.tensor_copy(kt[e][:, sc * 512:(sc + 1) * 512], ps)

        # ---- v projection into the space wk just freed ----
        v_pool = attn.enter_context(tc.tile_pool(name="v", bufs=1))
        vtt = v_pool.tile([128, 8, D], F32R, name="vtt")
        vt = [vtt[:, s] for s in range(8)]
        with ExitStack() as ph:
            psA2 = ph.enter_context(tc.tile_pool(name="psA2", bufs=6, space="PSUM"))
            for ec in range(2):
                for s_ in range(8):
                    ps = psA2.tile([128, 512], F32)
                    for d_ in range(8):
                        nc.tensor.matmul(
                            ps, lhsT=xk[:, d_, s_ * 128:(s_ + 1) * 128],
                            rhs=wv[:, d_, ec * 512:(ec + 1) * 512],
                            start=d_ == 0, stop=d_ == 7)
                    nc.vector.tensor_copy(vt[s_][:, ec * 512:(ec + 1) * 512], ps)

        # ---- attention over 512-query tiles, largest first. scores run at
        # N=512 so the 2-pass fp32r LDWEIGHTS stays hidden under the matmul
        # stream. A@V needs 8 psum banks for a 512-query out accumulation,
        # which does not fit next to the scores banks, so it runs as two
        # passes (q-halves) over the retained exp tiles. The dead xk tile is
        # reused as the exp-slot scratch and the dead wv tile holds eacc and
        # the diagonal masks. The denominator is one matmul per tile:
        # ones^T @ eacc -> [2, 512] in PSUM, DMA'd straight to dram.
        NU = S // 512
        dm_a = wv[:, 1, 0:512].bitcast(F32)
        dm_b = wv[:, 2, 0:512].bitcast(F32)
        idm = nc.sync.dma_start(out=wv[:, 1, 0:512], in_=dmask[0].bitcast(F32R))
        chain_to(nc.sync.dma_start(out=wv[:, 2, 0:512], in_=dmask[1].bitcast(F32R)), idm)
        ps_sc = attn.enter_context(tc.tile_pool(name="ps_sc", bufs=2, space="PSUM"))
        ps_out = attn.enter_context(tc.tile_pool(name="ps_out", bufs=1, space="PSUM"))
        ps_den = attn.enter_context(tc.tile_pool(name="ps_den", bufs=1, space="PSUM"))

        def av_pass(u, qs, jmax):
            """A@V + drain for q128 slices `qs`, k-blocks 0..jmax (q-outer,
            so each q's output drains while the next q's matmuls run)."""
            for q in qs:
                outp = [ps_out.tile([128, 512], F32, tag=f"po{q & 1}{ec}", name=f"po{q & 1}{ec}")
                        for ec in range(2)]
                for jj in range(jmax + 1):
                    for ec in range(2):
                        nc.tensor.matmul(
                            outp[ec], lhsT=xk[:, jj, q * 128:(q + 1) * 128],
                            rhs=vt[jj][:, ec * 512:(ec + 1) * 512],
                            start=jj == 0, stop=jj == jmax)
                row = u * 512 + q * 128
                osb = osb_pool.tile([128, D], F32, tag="osb", name="osb")
                nc.vector.tensor_copy(osb[:, 0:512], outp[0])
                nc.sync.dma_start(out=pout[row:row + 128, 0:256], in_=osb[:, 0:256])
                nc.sync.dma_start(out=pout[row:row + 128, 256:512], in_=osb[:, 256:512])
                nc.scalar.copy(osb[:, 512:1024], outp[1])
                nc.sync.dma_start(out=pout[row:row + 128, 512:768], in_=osb[:, 512:768])
                nc.sync.dma_start(out=pout[row:row + 128, 768:D], in_=osb[:, 768:D])

        for u in reversed(range(NU)):
            eacc = wv[:, 3 + (u & 1), 0:512]
            for jj in range(2 * u + 2):
                sp = ps_sc.tile([128, 512], F32)
                for e in range(8):
                    nc.tensor.matmul(
                        sp, lhsT=kt[e][:, jj * 128:(jj + 1) * 128],
                        rhs=qt[e][:, u * 512:(u + 1) * 512],
                        start=e == 0, stop=e == 7)
                if jj == 2 * u:
                    nc.vector.tensor_add(sp, sp, dm_a)
                elif jj == 2 * u + 1:
                    nc.vector.tensor_add(sp, sp, dm_b)
                et = xk[:, jj, 0:512]
                nc.scalar.activation(et, sp, EXP, scale=SCALE)
                if jj == 0:
                    nc.vector.tensor_copy(eacc, et)
                else:
                    nc.vector.tensor_add(eacc, eacc, et)
            av_pass(u, (0, 1), 2 * u)
            denp = ps_den.tile([128, 512], F32, tag=f"pd{u & 1}", name=f"pd{u & 1}")
            nc.tensor.matmul(denp[0:2, :], lhsT=ones, rhs=eacc,
                             start=True, stop=True)
            nc.scalar.copy(den_sb, denp[0:2, :])
            nc.sync.dma_start(out=den[0:1, u * 512:(u + 1) * 512],
                              in_=den_sb[0:1, :])
            av_pass(u, (2, 3), 2 * u + 1)

    nc.compile()
    return nc


def _prep_inputs(x, Wq, Wk, Wv):
    wqT = np.ascontiguousarray(Wq.T)
    wkT = np.ascontiguousarray(Wk.T)
    wvT = np.ascontiguousarray(Wv.T)
    kp = np.arange(128)[:, None]
    qc = np.arange(512)[None, :]
    in_maps = []
    for c in range(8):
        b, h = c // 2, c % 2
        xb = x[b]                                   # [S, D]
        if h:
            # swap adjacent 128-row blocks: query s -> slot s ^ 128
            xb = xb.reshape(8, 2, 128, D)[:, ::-1].reshape(S, D)
        xT = np.ascontiguousarray(xb.T)             # [D, S]
        # mask[kp, qc]: key offset within the 512-tile is 128h+kp (dm_a,
        # xk block 2u) or 256+128h+kp (dm_b, block 2u+1); query position
        # within the tile is qc ^ (128h) after the host-side pair swap.
        qpos = qc ^ (128 * h)
        dm_a = np.where(128 * h + kp <= qpos, np.float32(0.0), np.float32(-1e30))
        dm_b = np.where(256 + 128 * h + kp <= qpos, np.float32(0.0), np.float32(-1e30))
        dmask = np.stack([dm_a, dm_b]).astype(np.float32)
        in_maps.append({
            "xT": xT, "wqT": wqT, "wkT": wkT, "wvT": wvT,
            "dmask": np.ascontiguousarray(dmask),
        })
    return in_maps


def _unperm(a):
    """Undo the h=1 query pair swap on the leading (row) axis of length S."""
    return a.reshape(8, 2, 128, -1)[:, ::-1].reshape(S, -1)


def _run(inputs, trace=False, **kw):
    global _NC
    if _NC is None:
        _NC = _build()
    x = np.asarray(inputs["x"], dtype=np.float32)
    Wq = np.asarray(inputs["Wq"], dtype=np.float32)
    Wk = np.asarray(inputs["Wk"], dtype=np.float32)
    Wv = np.asarray(inputs["Wv"], dtype=np.float32)
    in_maps = _prep_inputs(x, Wq, Wk, Wv)
    res = bass_utils.run_bass_kernel_spmd(
        _NC, in_maps, core_ids=list(range(8)), trace=trace, **kw)
    out = np.empty((B, S, D), dtype=np.float32)
    for b in range(B):
        po = res.results[2 * b]["pout"] + _unperm(res.results[2 * b + 1]["pout"])
        dn = (res.results[2 * b]["den"].reshape(S, 1)
              + _unperm(res.results[2 * b + 1]["den"].reshape(S, 1)))
        out[b] = po / dn
    return out, res


def kernel(**inputs):
    out, _ = _run(inputs, trace=False)
    return out


# revision 30
# speedup vs baseline: 1.0327x; 1.0327x over previous
"""Causal attention (B=4, S=2048, D=1024, fp32) on 8 TRN2 NeuronCores.

Sharding: core c -> (batch b = c//2, key-parity h = c%2). Each core computes
q = x@Wq.T for all S queries of its batch, k/v only for key positions whose
128-block index has parity h (S/2 positions, causally load-balanced), then
scores^T = k q^T in [kpos, q] orientation (softmax denominator and A@V both
reduce over kpos = the PSUM contraction dim, so no on-chip transposes), and
returns the unnormalized partial output sum(exp(s)*v) plus the denominator
sum(exp(s)). Host adds the two partials per batch and divides. exp() is
computed without max-subtraction: scores*scale is ~N(0, 0.17) here, far from
fp32 overflow. All matmuls run as float32r (fp32 truncated inside the PE),
which streams at ~1 col/cycle warm for moving dims >= 256.

The parity-h key positions are extracted on-chip from the streamed x chunks
(no separate xk DMA). So the extraction slices are compile-time constants in
the shared SPMD NEFF, odd-parity cores receive x with adjacent 128-column
blocks swapped (query index XOR 128); their masks are built for the permuted
query order on the host and their pout/den rows are unpermuted on the host.
"""
import numpy as np

import concourse.bacc as bacc
import concourse.tile as tile
import concourse.mybir as mybir
from concourse import bass_utils
from concourse.tile import add_dep_helper
from contextlib import ExitStack

B, S, D = 4, 2048, 1024
QT = 256              # query tile
NT = S // QT          # 8 query tiles
SH = S // 2           # key positions per core
SCALE = 1.0 / 32.0    # 1/sqrt(D)
F32 = mybir.dt.float32
F32R = mybir.dt.float32r
BF16 = mybir.dt.bfloat16
EXP = mybir.ActivationFunctionType.Exp

_NC = None


def _dview(ap):
    """[D, C] dram tensor -> [128, 8, C] view (partition, d-block, col)."""
    return ap.rearrange("(d p) c -> p d c", p=128)


def _build():
    nc = bacc.Bacc()
    xT = nc.dram_tensor("xT", [D, S], F32, kind="ExternalInput").ap()
    wqT = nc.dram_tensor("wqT", [D, D], F32, kind="ExternalInput").ap()
    wkT = nc.dram_tensor("wkT", [D, D], F32, kind="ExternalInput").ap()
    wvT = nc.dram_tensor("wvT", [D, D], F32, kind="ExternalInput").ap()
    dmask = nc.dram_tensor("dmask", [2, 128, 512], F32, kind="ExternalInput").ap()
    pout = nc.dram_tensor("pout", [S, D], F32, kind="ExternalOutput").ap()
    den = nc.dram_tensor("den", [1, S], F32, kind="ExternalOutput").ap()

    def chain_to(inst, prev):
        add_dep_helper(inst.ins, prev.ins, sync=True, reason="input dma ordering")
        return inst

    with tile.TileContext(nc) as tc, ExitStack() as top:
        small = top.enter_context(tc.tile_pool(name="small", bufs=1))
        osb_pool = top.enter_context(tc.tile_pool(name="osb", bufs=2))
        qt_pool = top.enter_context(tc.tile_pool(name="qt", bufs=1))

        # q^T and k^T live as bf16: halves their SBUF footprint (so wk and
        # wv fit in fresh space with eager DMAs and no WAR stalls) and makes
        # the score matmuls 1-pass-LDWEIGHTS bf16 x bf16. The ~4e-3 relative
        # quantization on q/k perturbs the logits by ~1e-3 - far inside the
        # accuracy budget.
        qtt = qt_pool.tile([128, 8, S], BF16, name="qtt")
        qt = [qtt[:, e] for e in range(8)]
        ones_f = small.tile([128, 2], F32)
        ones = small.tile([128, 2], F32R)
        junk = small.tile([128, 512], F32R)
        den_sb = small.tile([2, 512], F32, name="den_sb")
        nc.vector.memset(ones_f, 1.0)
        nc.vector.tensor_copy(ones, ones_f)
        nc.vector.memset(junk.bitcast(F32), 0.0)
        nc.vector.tensor_copy(junk, junk)

        # xk ([d-part, d-block, parity-key-pos]) is filled from the streamed
        # x chunks during phase 1. wk and wv overlap the phase-1 pools'
        # lifetimes so the allocator gives them fresh space: their DMAs have
        # no WAR on the q-phase tiles and land during phase 1.
        attn = top.enter_context(ExitStack())
        xk_pool = attn.enter_context(tc.tile_pool(name="xk", bufs=1))
        xk = xk_pool.tile([128, 8, SH], F32R, name="xk")
        wk_pool = attn.enter_context(tc.tile_pool(name="wk", bufs=1))
        wk = wk_pool.tile([128, 8, D], F32R, name="wk")
        wv_pool = attn.enter_context(tc.tile_pool(name="wv", bufs=1))
        wv = wv_pool.tile([128, 8, D], F32R, name="wv")

        # ---- phase 1: q^T = Wq^T-contracted x^T, for all S queries ----
        # DMAs are emitted in consumption order on a single dependency
        # chain so HBM bandwidth follows the matmul stream: wq e0-slice +
        # x chunk 0 first (first PE group), then the wq tail paced against
        # the e-groups, then x chunks 1-3, then wk / wv / masks.
        with ExitStack() as ph:
            wq_pool = ph.enter_context(tc.tile_pool(name="wq", bufs=1))
            xs_pool = ph.enter_context(tc.tile_pool(name="xs", bufs=2))
            psB = ph.enter_context(tc.tile_pool(name="psB", bufs=6, space="PSUM"))
            warm_ps = ph.enter_context(tc.tile_pool(name="warm", bufs=1, space="PSUM"))
            wp = warm_ps.tile([128, 512], F32, name="wp")
            for _ in range(36):
                nc.tensor.matmul(wp[0:2, :], lhsT=junk[:, 0:2], rhs=junk,
                                 start=True, stop=True, skip_group_check=True)
            wq = wq_pool.tile([128, 8, D], F32R, name="wq")
            wqv = _dview(wqT.bitcast(F32R))
            xv = _dview(xT.bitcast(F32R))
            # strict-priority initial burst: only wq e0 + x chunk 0 (the
            # first PE group's inputs) race at t0; two chains hang off the
            # two x halves and deliver the wq tail, x chunks 1-3, and wk in
            # consumption order without stealing bandwidth from earlier
            # pieces.
            xs_tiles = [None] * (S // 512)
            xs_tiles[0] = xs_pool.tile([128, 8, 512], F32R, name="xs")
            # fine-grained initial burst: six 0.25 MiB transfers ramp six
            # queues in parallel so the first PE group's inputs (wq e0 +
            # x chunk 0) land as early as possible; the wq tail chains off
            # the x pieces and arrives well before the e4-e7 groups.
            nc.sync.dma_start(out=wq[:, :, 0:64], in_=wqv[:, :, 0:64])
            nc.sync.dma_start(out=wq[:, :, 64:128], in_=wqv[:, :, 64:128])
            x0 = [nc.sync.dma_start(out=xs_tiles[0][:, :, i * 128:(i + 1) * 128],
                                    in_=xv[:, :, i * 128:(i + 1) * 128])
                  for i in range(4)]
            x0a, x0b = x0[2], x0[3]
            wq1 = chain_to(nc.sync.dma_start(out=wq[:, :, 128:256], in_=wqv[:, :, 128:256]), x0[0])
            wq2 = chain_to(nc.sync.dma_start(out=wq[:, :, 256:512], in_=wqv[:, :, 256:512]), x0[1])
            wq3a = chain_to(nc.sync.dma_start(out=wq[:, :, 512:768], in_=wqv[:, :, 512:768]), x0[2])
            wq3b = chain_to(nc.sync.dma_start(out=wq[:, :, 768:D], in_=wqv[:, :, 768:D]), x0[3])
            xs_dma = [None] * (S // 512)
            pa, pb = wq3a, wq3b
            for c in range(1, S // 512):
                xs_tiles[c] = xs_pool.tile([128, 8, 512], F32R, name="xs")
                pa = chain_to(nc.sync.dma_start(
                    out=xs_tiles[c][:, :, 0:256],
                    in_=xv[:, :, c * 512:c * 512 + 256]), pa)
                pb = chain_to(nc.sync.dma_start(
                    out=xs_tiles[c][:, :, 256:512],
                    in_=xv[:, :, c * 512 + 256:(c + 1) * 512]), pb)
            iwk = chain_to(nc.sync.dma_start(
                out=wk, in_=_dview(wkT.bitcast(F32R))), pb)
            chain_to(nc.sync.dma_start(
                out=wv, in_=_dview(wvT.bitcast(F32R))), iwk)
            for c in range(S // 512):
                xs = xs_tiles[c]
                for e in range(8):
                    ps = psB.tile([128, 512], F32)
                    for d_ in range(8):
                        nc.tensor.matmul(ps, lhsT=wq[:, d_, e * 128:(e + 1) * 128],
                                         rhs=xs[:, d_, :], start=d_ == 0, stop=d_ == 7)
                    nc.vector.tensor_copy(qt[e][:, c * 512:(c + 1) * 512], ps)
                # extract this chunk's parity-h key positions (local column
                # blocks 0 and 2 after the host-side pair swap on h=1 cores)
                nc.scalar.copy(xk[:, :, c * 256:c * 256 + 128], xs[:, :, 0:128])
                nc.scalar.copy(xk[:, :, c * 256 + 128:c * 256 + 256], xs[:, :, 256:384])

        # ---- k projection; kt (bf16) goes into the freed wq/xs space ----
        kt_pool = attn.enter_context(tc.tile_pool(name="kt", bufs=1))
        ktt = kt_pool.tile([128, 8, SH], BF16, name="ktt")
        kt = [ktt[:, e] for e in range(8)]
        with ExitStack() as ph:
            psA = ph.enter_context(tc.tile_pool(name="psA", bufs=6, space="PSUM"))
            for sc in range(2):
                for e in range(8):
                    ps = psA.tile([128, 512], F32)
                    for d_ in range(8):
                        nc.tensor.matmul(
                            ps, lhsT=wk[:, d_, e * 128:(e + 1) * 128],
                            rhs=xk[:, d_, sc * 512:(sc + 1) * 512],
                            start=d_ == 0, stop=d_ == 7)
                    nc.vector.tensor_copy(kt[e][:, sc * 512:(sc + 1) * 512], ps)

        # ---- v projection into the space wk just freed ----
        v_pool = attn.enter_context(tc.tile_pool(name="v", bufs=1))
        vtt = v_pool.tile([128, 8, D], F32R, name="vtt")
        vt = [vtt[:, s] for s in range(8)]
        with ExitStack() as ph:
            psA2 = ph.enter_context(tc.tile_pool(name="psA2", bufs=6, space="PSUM"))
            for ec in range(2):
                for s_ in range(8):
                    ps = psA2.tile([128, 512], F32)
                    for d_ in range(8):
                        nc.tensor.matmul(
                            ps, lhsT=xk[:, d_, s_ * 128:(s_ + 1) * 128],
                            rhs=wv[:, d_, ec * 512:(ec + 1) * 512],
                            start=d_ == 0, stop=d_ == 7)
                    nc.vector.tensor_copy(vt[s_][:, ec * 512:(ec + 1) * 512], ps)

        # ---- attention over 512-query tiles, largest first. scores run at
        # N=512 so the 2-pass fp32r LDWEIGHTS stays hidden under the matmul
        # stream. A@V needs 8 psum banks for a 512-query out accumulation,
        # which does not fit next to the scores banks, so it runs as two
        # passes (q-halves) over the retained exp tiles. The dead xk tile is
        # reused as the exp-slot scratch and the dead wv tile holds eacc and
        # the diagonal masks. The denominator is one matmul per tile:
        # ones^T @ eacc -> [2, 512] in PSUM, DMA'd straight to dram.
        NU = S // 512
        dm_a = wv[:, 1, 0:512].bitcast(F32)
        dm_b = wv[:, 2, 0:512].bitcast(F32)
        idm = nc.sync.dma_start(out=wv[:, 1, 0:512], in_=dmask[0].bitcast(F32R))
        chain_to(nc.sync.dma_start(out=wv[:, 2, 0:512], in_=dmask[1].bitcast(F32R)), idm)
        ps_sc = attn.enter_context(tc.tile_pool(name="ps_sc", bufs=2, space="PSUM"))
        ps_out = attn.enter_context(tc.tile_pool(name="ps_out", bufs=1, space="PSUM"))
        ps_den = attn.enter_context(tc.tile_pool(name="ps_den", bufs=1, space="PSUM"))

        def av_pass(u, qs, jmax):
            """A@V + drain for q128 slices `qs`, k-blocks 0..jmax (q-outer,
            so each q's output drains while the next q's matmuls run)."""
            for q in qs:
                outp = [ps_out.tile([128, 512], F32, tag=f"po{q & 1}{ec}", name=f"po{q & 1}{ec}")
                        for ec in range(2)]
                for jj in range(jmax + 1):
                    for ec in range(2):
                        nc.tensor.matmul(
                            outp[ec], lhsT=xk[:, jj, q * 128:(q + 1) * 128],
                            rhs=vt[jj][:, ec * 512:(ec + 1) * 512],
                            start=jj == 0, stop=jj == jmax)
                row = u * 512 + q * 128
                osb = osb_pool.tile([128, D], F32, tag="osb", name="osb")
                nc.vector.tensor_copy(osb[:, 0:512], outp[0])
                nc.scalar.copy(osb[:, 512:1024], outp[1])
                nc.sync.dma_start(out=pout[row:row + 128, 0:512], in_=osb[:, 0:512])
                nc.sync.dma_start(out=pout[row:row + 128, 512:D], in_=osb[:, 512:D])

        for u in reversed(range(NU)):
            eacc = wv[:, 3 + (u & 1), 0:512]
            for jj in range(2 * u + 2):
                sp = ps_sc.tile([128, 512], F32)
                et = xk[:, jj, 0:512]
                if jj == 2 * u + 1:
                    # key block 2u+1 is fully masked for query cols [0,256)
                    # on both parities: compute only the upper 256 queries
                    # (N=256 stream) and zero the dead exp half.
                    for e in range(8):
                        nc.tensor.matmul(
                            sp[:, 0:256], lhsT=kt[e][:, jj * 128:(jj + 1) * 128],
                            rhs=qt[e][:, u * 512 + 256:(u + 1) * 512],
                            start=e == 0, stop=e == 7)
                    nc.vector.tensor_add(sp[:, 0:256], sp[:, 0:256], dm_b[:, 256:512])
                    nc.vector.memset(et[:, 0:256].bitcast(F32), 0.0)
                    nc.scalar.activation(et[:, 256:512], sp[:, 0:256], EXP, scale=SCALE)
                else:
                    for e in range(8):
                        nc.tensor.matmul(
                            sp, lhsT=kt[e][:, jj * 128:(jj + 1) * 128],
                            rhs=qt[e][:, u * 512:(u + 1) * 512],
                            start=e == 0, stop=e == 7)
                    if jj == 2 * u:
                        nc.vector.tensor_add(sp, sp, dm_a)
                    nc.scalar.activation(et, sp, EXP, scale=SCALE)
                if jj == 0:
                    nc.vector.tensor_copy(eacc, et)
                else:
                    nc.vector.tensor_add(eacc, eacc, et)
            av_pass(u, (0, 1), 2 * u)
            denp = ps_den.tile([128, 512], F32, tag=f"pd{u & 1}", name=f"pd{u & 1}")
            nc.tensor.matmul(denp[0:2, :], lhsT=ones, rhs=eacc,
                             start=True, stop=True)
            nc.scalar.copy(den_sb, denp[0:2, :])
            nc.sync.dma_start(out=den[0:1, u * 512:(u + 1) * 512],
                              in_=den_sb[0:1, :])
            av_pass(u, (2, 3), 2 * u + 1)

    nc.compile()
    return nc


def _prep_inputs(x, Wq, Wk, Wv):
    wqT = np.ascontiguousarray(Wq.T)
    wkT = np.ascontiguousarray(Wk.T)
    wvT = np.ascontiguousarray(Wv.T)
    kp = np.arange(128)[:, None]
    qc = np.arange(512)[None, :]
    in_maps = []
    for c in range(8):
        b, h = c // 2, c % 2
        xb = x[b]                                   # [S, D]
        if h:
            # swap adjacent 128-row blocks: query s -> slot s ^ 128
            xb = xb.reshape(8, 2, 128, D)[:, ::-1].reshape(S, D)
        xT = np.ascontiguousarray(xb.T)             # [D, S]
        # mask[kp, qc]: key offset within the 512-tile is 128h+kp (dm_a,
        # xk block 2u) or 256+128h+kp (dm_b, block 2u+1); query position
        # within the tile is qc ^ (128h) after the host-side pair swap.
        qpos = qc ^ (128 * h)
        dm_a = np.where(128 * h + kp <= qpos, np.float32(0.0), np.float32(-1e30))
        dm_b = np.where(256 + 128 * h + kp <= qpos, np.float32(0.0), np.float32(-1e30))
        dmask = np.stack([dm_a, dm_b]).astype(np.float32)
        in_maps.append({
            "xT": xT, "wqT": wqT, "wkT": wkT, "wvT": wvT,
            "dmask": np.ascontiguousarray(dmask),
        })
    return in_maps


def _unperm(a):
    """Undo the h=1 query pair swap on the leading (row) axis of length S."""
    return a.reshape(8, 2, 128, -1)[:, ::-1].reshape(S, -1)


def _run(inputs, trace=False, **kw):
    global _NC
    if _NC is None:
        _NC = _build()
    x = np.asarray(inputs["x"], dtype=np.float32)
    Wq = np.asarray(inputs["Wq"], dtype=np.float32)
    Wk = np.asarray(inputs["Wk"], dtype=np.float32)
    Wv = np.asarray(inputs["Wv"], dtype=np.float32)
    in_maps = _prep_inputs(x, Wq, Wk, Wv)
    res = bass_utils.run_bass_kernel_spmd(
        _NC, in_maps, core_ids=list(range(8)), trace=trace, **kw)
    out = np.empty((B, S, D), dtype=np.float32)
    for b in range(B):
        po = res.results[2 * b]["pout"] + _unperm(res.results[2 * b + 1]["pout"])
        dn = (res.results[2 * b]["den"].reshape(S, 1)
              + _unperm(res.results[2 * b + 1]["den"].reshape(S, 1)))
        out[b] = po / dn
    return out, res


def kernel(**inputs):
    out, _ = _run(inputs, trace=False)
    return out
